# revision 8
# baseline (speedup 1.0000x reference)
"""EnhancedAdaptiveLoRAPooling fused kernel for 8x Trainium2 NeuronCores.

Strategy (data-parallel over batch, v3 = fp16 streaming + engine balance):
  - hidden_states [8, 4096, 768] sharded by batch element: core i gets
    x_i [4096, 768], packed on host to fp16 [8, 128, 6, 512]
    (k = 512-token chunk, partition p, hidden chunk c, token t) so every
    DMA partition line is 6 KiB contiguous.  fp16 transport halves HBM
    traffic both ways (rel-err budget is 2e-2; fp16 path measures ~7e-4).
  - All routing math (cosine/euclid sims, 4-layer similarity MLP, top-3
    selection + thresholding, weighted LoRA pooling, fusion weights) is
    computed on-device, replicated on every core.
  - The two LoRA branches are fused into one rank-16 LoRA via
       u = G2.T @ (laG @ x),  y = x + B_comb.T @ u
    where G2 folds top-3 weights + fusion coefficients and B_comb is the
    pooled B-side assembled on device.
  - Engine balance per 512-token body: chunks 0-3 residual-add on DVE,
    chunks 4-5 via PE identity-accumulate (x added in PSUM) + ACT copy,
    so no single engine paces the pipeline.
  - PE p-state: dummy matmuls at t~1us keep the PE clock ramped through
    const-DMA arrival; ACT tables are limited to 2 loads, both prefetched
    off the critical path.  W1/comb run fp8e4 (sims margin is ~7x).
  - Memory-bound: ~6.3 MiB in + 6.3 MiB out + ~1.9 MiB consts per core.
"""

import numpy as np

B, S, H = 8, 4096, 768
N_TASKS, R = 16, 8
SCALING = 2.0
NCORES = 8
TPC = (B * S) // NCORES          # tokens per core = 4096
CK = 512                         # token chunk (one PSUM bank wide)
NK = TPC // CK                   # 8 chunks per core
NCH = H // 128                   # 6 hidden chunks
NR = N_TASKS * R                 # 128 = (task, rank) pairs
N_WARM_MM = 26                   # PE p-state warmup matmuls

# blobR f32 cols: teT 96 | curT 6 | W4T 1 | M8 16 | oh 1 | bf16: W2T 512,
# W3T 128 | fp8: combT 48
_R_COLS = 120 + 640 + 48
# blobL f32 cols (fp16 payload): laGT 384 | lbG 384 | ident128 64
_L_COLS = 832
# blob2 f32 cols (rows = 16)
_B2_COLS = 768 + 128 + 16 + 768 + 16 + 128 + 457

_PROGRAM = None


def _build_program():
    from contextlib import ExitStack

    import concourse.bass as bass  # noqa: F401
    import concourse.tile as tile
    from concourse import bacc, mybir

    f32 = mybir.dt.float32
    f16 = mybir.dt.float16
    bf16 = mybir.dt.bfloat16
    f8 = mybir.dt.float8e4
    AF = mybir.ActivationFunctionType
    OP = mybir.AluOpType
    AX = mybir.AxisListType

    nc = bacc.Bacc("TRN2", target_bir_lowering=False, debug=False)

    xin = nc.dram_tensor("xin", [NK, 128, NCH, CK], f16, kind="ExternalInput").ap()
    blobR = nc.dram_tensor("blobR", [128, _R_COLS], f32, kind="ExternalInput").ap()
    blobW1 = nc.dram_tensor("blobW1", [128, 1536], f32, kind="ExternalInput").ap()
    blobL = nc.dram_tensor("blobL", [128, _L_COLS], f32, kind="ExternalInput").ap()
    blob2 = nc.dram_tensor("blob2", [16, _B2_COLS], f32, kind="ExternalInput").ap()
    yout = nc.dram_tensor("yout", [NK, 128, NCH, CK], f16, kind="ExternalOutput").ap()

    with tile.TileContext(nc) as tc:
        with ExitStack() as ctx:
            const = ctx.enter_context(tc.tile_pool(name="const", bufs=1))
            pers = ctx.enter_context(tc.tile_pool(name="pers", bufs=1))
            xp = ctx.enter_context(tc.tile_pool(name="xp", bufs=NK))
            vp = ctx.enter_context(tc.tile_pool(name="vp", bufs=1, space="PSUM"))
            vsb = ctx.enter_context(tc.tile_pool(name="vsb", bufs=NK))

            pro = ExitStack()
            pp = pro.enter_context(tc.tile_pool(name="pp", bufs=3, space="PSUM"))
            wps = pro.enter_context(tc.tile_pool(name="wps", bufs=1, space="PSUM"))

            # ---- engine warmup (before consts land) ----
            # DVE zeroes a dummy tile; PE chews matmuls on it so the PE
            # clock is fully ramped when real work arrives; ACT preloads
            # the sqrt table (covers sqrt/square/relu/copy).
            wtile = pers.tile([128, CK], f16, name="wtile")
            nc.vector.memset(wtile, 0.0)
            wsc = pers.tile([1, 4], f32, name="wsc")
            nc.scalar.activation(wsc, wtile[:1, :4], AF.Sqrt)

            # ---- const loads ----
            blobR_sb = const.tile([128, _R_COLS], f32, name="blobR_sb")
            nc.sync.dma_start(out=blobR_sb, in_=blobR)
            blob2_sb = const.tile([16, _B2_COLS], f32, name="blob2_sb")
            nc.sync.dma_start(out=blob2_sb, in_=blob2)
            blobW1_sb = const.tile([128, 1536], f32, name="blobW1_sb")
            nc.scalar.dma_start(out=blobW1_sb, in_=blobW1)
            blobL_sb = const.tile([128, _L_COLS], f32, name="blobL_sb")
            nc.scalar.dma_start(out=blobL_sb, in_=blobL)

            teT_sb = blobR_sb[:, 0:96].rearrange("p (c j) -> p c j", c=6)
            curT_sb = blobR_sb[:, 96:102].rearrange("p (c j) -> p c j", c=6)
            W4T_sb = blobR_sb[:, 102:103]
            M8_sb = blobR_sb[:, 103:119]
            oh_sb = blobR_sb[:, 119:120]
            W2T_sb = blobR_sb[:, 120:632].bitcast(bf16).rearrange(
                "p (c j) -> p c j", c=4)
            W3T_sb = blobR_sb[:, 632:760].bitcast(bf16).rearrange(
                "p (c j) -> p c j", c=2)
            combT_sb = blobR_sb[:, 760:808].bitcast(f8).rearrange(
                "p (c j) -> p c j", c=12)
            W1T_sb = blobW1_sb.bitcast(f8).rearrange("p (c j) -> p c j", c=12)
            laGT_sb = blobL_sb[:, 0:384].bitcast(f16).rearrange(
                "p (c j) -> p c j", c=6)
            lbG_sb = blobL_sb[:, 384:768].bitcast(f16)
            ident128_sb = blobL_sb[:, 768:832].bitcast(f16)

            te_row_sb = blob2_sb[:, 0:768]
            E16_sb = blob2_sb[:, 768:896]
            ident_sb = blob2_sb[:, 896:912]
            cur_row_sb = blob2_sb[:1, 912:1680]
            ones16_sb = blob2_sb[:1, 1680:1696]
            ones128_sb = blob2_sb[:1, 1696:1824]
            ones16b_sb = blob2_sb[:1, 1824:1832].bitcast(bf16)
            b1_sb = blob2_sb[:1, 1832:2088].bitcast(bf16)
            b2_sb = blob2_sb[:1, 2088:2216].bitcast(bf16)
            b3_sb = blob2_sb[:1, 2216:2280].bitcast(bf16)
            b4_sb = blob2_sb[:1, 2280:2281].bitcast(bf16)[:, 0:1]

            # ---- x-in DMAs (8 chunk DMAs on the sync ring, after consts) ----
            xhs = []
            for k in range(NK):
                xh = xp.tile([128, NCH, CK], f16, tag="xh", name=f"xh{k}")
                nc.sync.dma_start(out=xh, in_=xin[k])
                xhs.append(xh)

            # ---- PE p-state warmup on the dummy tile ----
            w_ps = wps.tile([16, CK], f32, tag="wps", name="w_ps")
            for _ in range(N_WARM_MM):
                nc.tensor.matmul(w_ps, lhsT=wtile[:, 0:16], rhs=wtile,
                                 start=True, stop=True)

            v_sbs = {}

            def emit_v(k):
                v_ps = vp.tile([128, CK], f32, tag="v", name="v_ps")
                for c in range(NCH):
                    nc.tensor.matmul(v_ps, lhsT=laGT_sb[:, c, :],
                                     rhs=xhs[k][:, c, :],
                                     start=(c == 0), stop=(c == NCH - 1))
                v_sb = vsb.tile([128, CK], f16, tag="v_sb", name=f"v{k}")
                nc.scalar.copy(v_sb, v_ps)
                v_sbs[k] = v_sb

            # ================= routing prologue (replicated) =================
            # dots[n] = te[n] . cur
            dots_ps = pp.tile([16, 1], f32, tag="pp")
            for c in range(NCH):
                nc.tensor.matmul(dots_ps, lhsT=teT_sb[:, c, :], rhs=curT_sb[:, c, :],
                                 start=(c == 0), stop=(c == NCH - 1))
            dots = pers.tile([16, 1], f32)
            nc.scalar.copy(dots, dots_ps)

            # similarity MLP layer 1 (fp8)
            h1_ps = pp.tile([16, 512], f32, tag="pp")
            for c in range(12):
                nc.tensor.matmul(h1_ps, lhsT=combT_sb[:, c, :], rhs=W1T_sb[:, c, :],
                                 start=(c == 0), stop=False)
            nc.tensor.matmul(h1_ps, lhsT=ones16b_sb, rhs=b1_sb, start=False, stop=True)
            h1 = pers.tile([16, 512], f32)
            nc.scalar.activation(h1, h1_ps, AF.Relu)
            nc.scalar.activation(wsc[:1, :1], h1[:1, 0:1], AF.Sigmoid)

            # norms (DVE, keeps ACT free) + cur2 broadcast
            scr_te = pers.tile([16, H], f32)
            te2 = pers.tile([16, 1], f32)
            nc.vector.scalar_tensor_tensor(scr_te, in0=te_row_sb, scalar=1.0,
                                           in1=te_row_sb, op0=OP.mult, op1=OP.mult,
                                           accum_out=te2)
            scr_cur = pers.tile([1, H], f32)
            cur2 = pers.tile([1, 1], f32)
            nc.vector.scalar_tensor_tensor(scr_cur, in0=cur_row_sb, scalar=1.0,
                                           in1=cur_row_sb, op0=OP.mult, op1=OP.mult,
                                           accum_out=cur2)
            c2b_ps = pp.tile([16, 1], f32, tag="pp")
            nc.tensor.matmul(c2b_ps, lhsT=ones16_sb, rhs=cur2, start=True, stop=True)
            c2b = pers.tile([16, 1], f32)
            nc.scalar.copy(c2b, c2b_ps)

            # euclid distance pieces on DVE (needs dots/te2/c2b only)
            e2 = pers.tile([16, 1], f32)
            nc.vector.scalar_tensor_tensor(e2, in0=dots, scalar=-2.0, in1=te2,
                                           op0=OP.mult, op1=OP.add)
            nc.vector.tensor_add(e2, e2, c2b)
            nc.vector.tensor_scalar_max(e2, e2, 0.0)

            h1T = pers.tile([128, 4, 16], bf16)
            for c in range(4):
                tr_ps = pp.tile([128, 16], f32, tag="pp")
                nc.tensor.transpose(tr_ps, h1[:, c * 128:(c + 1) * 128], ident_sb)
                nc.scalar.copy(h1T[:, c, :], tr_ps)
            h2_ps = pp.tile([16, 256], f32, tag="pp")
            for c in range(4):
                nc.tensor.matmul(h2_ps, lhsT=h1T[:, c, :], rhs=W2T_sb[:, c, :],
                                 start=(c == 0), stop=False)
            nc.tensor.matmul(h2_ps, lhsT=ones16b_sb, rhs=b2_sb, start=False, stop=True)
            h2 = pers.tile([16, 256], f32)
            nc.scalar.activation(h2, h2_ps, AF.Relu)
            h2T = pers.tile([128, 2, 16], bf16)
            for c in range(2):
                tr_ps = pp.tile([128, 16], f32, tag="pp")
                nc.tensor.transpose(tr_ps, h2[:, c * 128:(c + 1) * 128], ident_sb)
                nc.scalar.copy(h2T[:, c, :], tr_ps)
            h3_ps = pp.tile([16, 128], f32, tag="pp")
            for c in range(2):
                nc.tensor.matmul(h3_ps, lhsT=h2T[:, c, :], rhs=W3T_sb[:, c, :],
                                 start=(c == 0), stop=False)
            nc.tensor.matmul(h3_ps, lhsT=ones16b_sb, rhs=b3_sb, start=False, stop=True)
            h3 = pers.tile([16, 128], f32)
            nc.scalar.activation(h3, h3_ps, AF.Relu)
            h3T = pers.tile([128, 16], f32)
            tr_ps = pp.tile([128, 16], f32, tag="pp")
            nc.tensor.transpose(tr_ps, h3, ident_sb)
            nc.scalar.copy(h3T, tr_ps)

            # sqrt block (sqrt table also covers relu/square/copy; the one
            # sigmoid table load is prefetched via the h1-gated dummy above)
            emb_n = pers.tile([16, 1], f32)
            nc.scalar.sqrt(emb_n, te2)
            curn16 = pers.tile([16, 1], f32)
            nc.scalar.sqrt(curn16, c2b)
            curn = pers.tile([1, 1], f32)
            nc.scalar.sqrt(curn, cur2)
            eu = pers.tile([16, 1], f32)
            nc.scalar.sqrt(eu, e2)
            den = pers.tile([16, 1], f32)
            nc.vector.tensor_mul(den, emb_n, curn16)
            nc.vector.tensor_scalar_max(den, den, 1e-8)
            rden = pers.tile([16, 1], f32)
            nc.vector.reciprocal(rden, den)
            eup1 = pers.tile([16, 1], f32)
            nc.vector.tensor_scalar_add(eup1, eu, 1.0)

            z4_ps = pp.tile([16, 1], f32, tag="pp")
            nc.tensor.matmul(z4_ps, lhsT=h3T, rhs=W4T_sb, start=True, stop=False)
            nc.tensor.matmul(z4_ps, lhsT=ones16b_sb, rhs=b4_sb, start=False, stop=True)
            nn_sim = pers.tile([16, 1], f32)
            nc.scalar.activation(nn_sim, z4_ps, AF.Sigmoid)
            emit_v(0)

            # cos / euclid sims (DVE)
            cos = pers.tile([16, 1], f32)
            nc.vector.tensor_mul(cos, dots, rden)
            es = pers.tile([16, 1], f32)
            nc.vector.reciprocal(es, eup1)

            sims16 = pers.tile([16, 1], f32)
            nc.vector.scalar_tensor_tensor(sims16, in0=cos, scalar=0.4 / 0.3, in1=es,
                                           op0=OP.mult, op1=OP.add)
            nc.vector.tensor_add(sims16, sims16, nn_sim)
            nc.vector.tensor_scalar_mul(sims16, sims16, 0.3)
            sr_ps = pp.tile([1, 16], f32, tag="pp")
            nc.tensor.transpose(sr_ps, sims16, ident_sb)
            sims_row = pers.tile([1, 16], f32)
            nc.scalar.copy(sims_row, sr_ps)
            emit_v(1)

            # ---- top-3 threshold (DVE; overlaps v on PE) ----
            m1 = pers.tile([1, 1], f32)
            nc.vector.reduce_max(m1, sims_row, axis=AX.X)
            msk = pers.tile([1, 16], f32)
            nc.vector.tensor_scalar(msk, in0=sims_row, scalar1=m1, scalar2=None, op0=OP.is_ge)
            s2 = pers.tile([1, 16], f32)
            nc.vector.scalar_tensor_tensor(s2, in0=msk, scalar=-1e30, in1=sims_row,
                                           op0=OP.mult, op1=OP.add)
            m2 = pers.tile([1, 1], f32)
            nc.vector.reduce_max(m2, s2, axis=AX.X)
            msk2 = pers.tile([1, 16], f32)
            nc.vector.tensor_scalar(msk2, in0=s2, scalar1=m2, scalar2=None, op0=OP.is_ge)
            s3 = pers.tile([1, 16], f32)
            nc.vector.scalar_tensor_tensor(s3, in0=msk2, scalar=-1e30, in1=s2,
                                           op0=OP.mult, op1=OP.add)
            m3 = pers.tile([1, 1], f32)
            nc.vector.reduce_max(m3, s3, axis=AX.X)
            ge3 = pers.tile([1, 16], f32)
            nc.vector.tensor_scalar(ge3, in0=sims_row, scalar1=m3, scalar2=None, op0=OP.is_ge)
            pos = pers.tile([1, 16], f32)
            nc.vector.tensor_scalar(pos, in0=sims_row, scalar1=0.0, scalar2=None, op0=OP.is_gt)
            m12 = pers.tile([1, 16], f32)
            nc.vector.tensor_mul(m12, ge3, pos)
            w_row = pers.tile([1, 16], f32)
            total = pers.tile([1, 1], f32)
            nc.vector.scalar_tensor_tensor(w_row, in0=m12, scalar=1.0, in1=sims_row,
                                           op0=OP.mult, op1=OP.mult, accum_out=total)
            tpos = pers.tile([1, 1], f32)
            nc.vector.tensor_scalar(tpos, in0=total, scalar1=0.0, scalar2=None, op0=OP.is_gt)
            tm1 = pers.tile([1, 1], f32)
            nc.vector.tensor_scalar_add(tm1, total, -1.0)
            safe = pers.tile([1, 1], f32)
            nc.vector.scalar_tensor_tensor(safe, in0=tm1, scalar=tpos, in1=ones16_sb[:, 0:1],
                                           op0=OP.mult, op1=OP.add)
            rinv = pers.tile([1, 1], f32)
            nc.vector.reciprocal(rinv, safe)
            wn_row = pers.tile([1, 16], f32)
            nc.vector.tensor_scalar_mul(wn_row, w_row, rinv)

            # fusion coefficients
            fw = pers.tile([1, 1], f32)
            nc.vector.tensor_scalar(fw, in0=curn, scalar1=0.1, scalar2=0.5,
                                    op0=OP.mult, op1=OP.min)
            cc = pers.tile([1, 2], f32)   # [c2*S | c1*S]
            c2v = pers.tile([1, 1], f32)
            nc.vector.tensor_mul(c2v, fw, tpos)
            nc.vector.tensor_scalar_mul(cc[:, 0:1], c2v, SCALING)
            nc.vector.tensor_scalar(cc[:, 1:2], in0=cc[:, 0:1], scalar1=-1.0, scalar2=SCALING,
                                    op0=OP.mult, op1=OP.add)
            ccb_ps = pp.tile([128, 2], f32, tag="pp")
            nc.tensor.matmul(ccb_ps, lhsT=ones128_sb, rhs=cc, start=True, stop=True)
            cc_b = pers.tile([128, 2], f32)
            nc.scalar.copy(cc_b, ccb_ps)

            # wn onto 128 (task,rank) partitions + spread along free dim
            wc_ps = pp.tile([16, 1], f32, tag="pp")
            nc.tensor.transpose(wc_ps, wn_row, ident_sb[:1, :1])
            wn_col = pers.tile([16, 1], f32)
            nc.scalar.copy(wn_col, wc_ps)
            we_ps = pp.tile([128, 1], f32, tag="pp")
            nc.tensor.matmul(we_ps, lhsT=E16_sb, rhs=wn_col, start=True, stop=True)
            wn_ext = pers.tile([128, 1], f32)
            nc.scalar.copy(wn_ext, we_ps)
            # selectors: G2 [128,16] (A-side, scaled) and sc_a (B-side lhsT)
            sc_a = pers.tile([128, 16], f16)
            nc.vector.tensor_scalar_mul(sc_a[:, 0:8], M8_sb[:, 0:8], oh_sb)
            nc.vector.tensor_scalar_mul(sc_a[:, 8:16], M8_sb[:, 8:16], wn_ext)
            G2f = pers.tile([128, 16], f16)
            nc.vector.tensor_scalar(G2f[:, 0:8], in0=sc_a[:, 0:8], scalar1=cc_b[:, 1:2],
                                    scalar2=None, op0=OP.mult)
            nc.vector.tensor_scalar(G2f[:, 8:16], in0=sc_a[:, 8:16], scalar1=cc_b[:, 0:1],
                                    scalar2=None, op0=OP.mult)

            bc_ps = pp.tile([16, H], f32, tag="bc", bufs=1)
            nc.tensor.matmul(bc_ps[:, 0:512], lhsT=sc_a, rhs=lbG_sb[:, 0:512],
                             start=True, stop=True)
            nc.tensor.matmul(bc_ps[:, 512:768], lhsT=sc_a, rhs=lbG_sb[:, 512:768],
                             start=True, stop=True)
            B_comb = pers.tile([16, H], f16)
            nc.scalar.copy(B_comb, bc_ps)

            pro.close()

            # ================= main loop =================
            with (
                tc.tile_pool(name="yp", bufs=2) as yp,
                tc.tile_pool(name="usb", bufs=3) as usb,
                tc.tile_pool(name="ups", bufs=2, space="PSUM") as ups,
                tc.tile_pool(name="lps", bufs=5, space="PSUM") as lps,
            ):
                u_sbs = {}

                def emit_u(k):
                    u_ps = ups.tile([16, CK], f32, tag="ups", name="u_ps")
                    nc.tensor.matmul(u_ps, lhsT=G2f, rhs=v_sbs[k],
                                     start=True, stop=True)
                    u_sb = usb.tile([16, CK], f16, tag="usb", name="u_sb")
                    nc.scalar.copy(u_sb, u_ps)
                    u_sbs[k] = u_sb

                def emit_lora(k):
                    yt = yp.tile([128, NCH, CK], f16, tag="yt", name="yt")
                    for c in range(NCH):
                        l_ps = lps.tile([128, CK], f32, tag="lora", name="l_ps")
                        if c < 4:
                            nc.tensor.matmul(l_ps,
                                             lhsT=B_comb[:, c * 128:(c + 1) * 128],
                                             rhs=u_sbs[k], start=True, stop=True)
                            nc.vector.tensor_add(yt[:, c, :], xhs[k][:, c, :], l_ps)
                        else:
                            # x folded in on the PE; ACT does the PSUM drain
                            nc.tensor.matmul(l_ps,
                                             lhsT=B_comb[:, c * 128:(c + 1) * 128],
                                             rhs=u_sbs[k], start=True, stop=False)
                            nc.tensor.matmul(l_ps, lhsT=ident128_sb,
                                             rhs=xhs[k][:, c, :],
                                             start=False, stop=True)
                            nc.scalar.copy(yt[:, c, :], l_ps)
                    oeng = nc.scalar if k % 2 == 0 else nc.gpsimd
                    oeng.dma_start(out=yout[k], in_=yt)

                emit_u(0)
                emit_u(1)
                emit_lora(0)
                emit_v(2)
                emit_u(2)
                emit_lora(1)
                emit_v(3)
                emit_u(3)
                emit_lora(2)
                emit_v(4)
                emit_u(4)
                emit_lora(3)
                emit_v(5)
                emit_u(5)
                emit_lora(4)
                emit_v(6)
                emit_u(6)
                emit_lora(5)
                emit_v(7)
                emit_u(7)
                emit_lora(6)
                emit_lora(7)

    nc.compile()
    return nc


def _get_program():
    global _PROGRAM
    if _PROGRAM is None:
        _PROGRAM = _build_program()
    return _PROGRAM


def _chunkpack(a):
    # [C*128, J] -> [128, C*J] so blob[p, c*J+j] = a[c*128+p, j]
    C = a.shape[0] // 128
    return a.reshape(C, 128, -1).transpose(1, 0, 2).reshape(128, -1)


def _make_in_maps(inputs):
    import ml_dtypes

    bfd = ml_dtypes.bfloat16
    f8d = ml_dtypes.float8_e4m3fn

    def bfpack(a):
        return np.ascontiguousarray(a.astype(bfd)).view(np.float32)

    def f16pack(a):
        return np.ascontiguousarray(a.astype(np.float16)).view(np.float32)

    def f8pack(a):
        return np.ascontiguousarray(a.astype(f8d)).view(np.float32)

    hs = np.ascontiguousarray(np.asarray(inputs["hidden_states"], np.float32))
    cur = np.ascontiguousarray(np.asarray(inputs["task_embedding"], np.float32))
    la = np.ascontiguousarray(np.asarray(inputs["loras_a"], np.float32))
    lb = np.ascontiguousarray(np.asarray(inputs["loras_b"], np.float32))
    te = np.ascontiguousarray(np.asarray(inputs["task_embeds"], np.float32))
    W1 = np.asarray(inputs["W1"], np.float32)
    W2 = np.asarray(inputs["W2"], np.float32)
    W3 = np.asarray(inputs["W3"], np.float32)
    W4 = np.asarray(inputs["W4"], np.float32)
    b1 = np.asarray(inputs["b1"], np.float32)
    b2 = np.asarray(inputs["b2"], np.float32)
    b3 = np.asarray(inputs["b3"], np.float32)
    b4 = np.asarray(inputs["b4"], np.float32)
    tid = int(np.asarray(inputs["current_task_id"]))

    idx = np.arange(NR)
    n_idx, r_idx = idx // R, idx % R
    M8 = np.zeros((NR, N_TASKS), np.float32)
    for j in range(N_TASKS):
        M8[:, j] = (r_idx == (j % R)).astype(np.float32)
    E16 = np.zeros((N_TASKS, NR), np.float32)
    E16[n_idx, idx] = 1.0
    onehot_ext = (n_idx == tid).astype(np.float32).reshape(NR, 1)

    comb = np.concatenate([np.repeat(cur[:, None], N_TASKS, axis=1), te.T], axis=0)
    blobR = np.concatenate([
        _chunkpack(np.ascontiguousarray(te.T)),               # 96  teT
        cur.reshape(6, 128).T,                                # 6   curT
        np.ascontiguousarray(W4.T),                           # 1   W4T
        M8,                                                   # 16
        onehot_ext,                                           # 1
        bfpack(_chunkpack(np.ascontiguousarray(W2.T))),       # 512 W2T (bf16)
        bfpack(_chunkpack(np.ascontiguousarray(W3.T))),       # 128 W3T (bf16)
        f8pack(_chunkpack(comb)),                             # 48  combT (fp8)
    ], axis=1).astype(np.float32)
    assert blobR.shape == (128, _R_COLS), blobR.shape

    blobW1 = f8pack(_chunkpack(np.ascontiguousarray(W1.T)))
    assert blobW1.shape == (128, 1536), blobW1.shape

    blobL = np.concatenate([
        f16pack(_chunkpack(np.ascontiguousarray(la.reshape(NR, H).T))),  # 384
        f16pack(lb.transpose(0, 2, 1).reshape(NR, H)),        # 384 lbG
        f16pack(np.eye(128, dtype=np.float32)),               # 64  ident128
    ], axis=1).astype(np.float32)
    assert blobL.shape == (128, _L_COLS), blobL.shape

    def row0(a, n):
        b = np.zeros((16, n), np.float32)
        b[0, :] = a.reshape(-1)
        return b

    bfrow = np.concatenate([
        np.ones(16, np.float32), b1, b2, b3, b4, np.zeros(1, np.float32)])
    bfrow = np.ascontiguousarray(bfrow.astype(bfd)).view(np.float32)
    blob2 = np.concatenate([
        te,                                                   # 768
        E16,                                                  # 128
        np.eye(16, dtype=np.float32),                         # 16
        row0(cur, 768),
        row0(np.ones(16, np.float32), 16),
        row0(np.ones(NR, np.float32), 128),
        row0(bfrow, 457),                                     # bf16: ones|b1..b4
    ], axis=1).astype(np.float32)
    assert blob2.shape == (16, _B2_COLS), blob2.shape

    rep = {"blobR": blobR, "blobW1": blobW1, "blobL": blobL, "blob2": blob2}

    x2 = hs.reshape(B * S, H)
    in_maps = []
    for i in range(NCORES):
        shard = x2[i * TPC:(i + 1) * TPC]                     # [TPC, H]
        xpk = shard.reshape(NK, CK, NCH, 128).transpose(0, 3, 2, 1)
        in_maps.append({"xin": np.ascontiguousarray(xpk.astype(np.float16)),
                        **rep})
    return in_maps


def _unpack_core_y(yarr):
    # [NK, 128, NCH, CK] fp16 -> [TPC, H] f32
    return np.ascontiguousarray(
        yarr.transpose(0, 3, 2, 1).astype(np.float32)).reshape(TPC, H)


def kernel(**inputs):
    from concourse.bass_utils import run_bass_kernel_spmd

    nc = _get_program()
    in_maps = _make_in_maps(inputs)
    res = run_bass_kernel_spmd(nc, in_maps, core_ids=list(range(NCORES)))
    out = np.empty((B * S, H), np.float32)
    for i, r in enumerate(res.results):
        out[i * TPC:(i + 1) * TPC] = _unpack_core_y(r["yout"])
    return out.reshape(B, S, H)


# revision 9
# speedup vs baseline: 1.0202x; 1.0202x over previous
"""EnhancedAdaptiveLoRAPooling fused kernel for 8x Trainium2 NeuronCores.

Strategy (data-parallel over batch, v3 = fp16 streaming + engine balance):
  - hidden_states [8, 4096, 768] sharded by batch element: core i gets
    x_i [4096, 768], packed on host to fp16 [8, 128, 6, 512]
    (k = 512-token chunk, partition p, hidden chunk c, token t) so every
    DMA partition line is 6 KiB contiguous.  fp16 transport halves HBM
    traffic both ways (rel-err budget is 2e-2; fp16 path measures ~7e-4).
  - All routing math (cosine/euclid sims, 4-layer similarity MLP, top-3
    selection + thresholding, weighted LoRA pooling, fusion weights) is
    computed on-device, replicated on every core.
  - The two LoRA branches are fused into one rank-16 LoRA via
       u = G2.T @ (laG @ x),  y = x + B_comb.T @ u
    where G2 folds top-3 weights + fusion coefficients and B_comb is the
    pooled B-side assembled on device.
  - Engine balance per 512-token body: chunks 0-3 residual-add on DVE,
    chunks 4-5 via PE identity-accumulate (x added in PSUM) + ACT copy,
    so no single engine paces the pipeline.
  - PE p-state: dummy matmuls at t~1us keep the PE clock ramped through
    const-DMA arrival; ACT tables are limited to 2 loads, both prefetched
    off the critical path.  W1/comb run fp8e4 (sims margin is ~7x).
  - Memory-bound: ~6.3 MiB in + 6.3 MiB out + ~1.9 MiB consts per core.
"""

import numpy as np

B, S, H = 8, 4096, 768
N_TASKS, R = 16, 8
SCALING = 2.0
NCORES = 8
TPC = (B * S) // NCORES          # tokens per core = 4096
CK = 512                         # token chunk (one PSUM bank wide)
NK = TPC // CK                   # 8 chunks per core
NCH = H // 128                   # 6 hidden chunks
NR = N_TASKS * R                 # 128 = (task, rank) pairs
N_WARM_MM = 26                   # PE p-state warmup matmuls

# blobR f32 cols: teT 96 | curT 6 | W4T 1 | M8 16 | oh 1 | bf16: W2T 512,
# W3T 128 | fp8: combT 48
_R_COLS = 120 + 640 + 48
# blobL f32 cols (fp16 payload): laGT 384 | lbG 384 | ident128 64
_L_COLS = 832
# blob2 f32 cols (rows = 16)
_B2_COLS = 768 + 128 + 16 + 768 + 16 + 128 + 457

_PROGRAM = None


def _build_program():
    from contextlib import ExitStack

    import concourse.bass as bass  # noqa: F401
    import concourse.tile as tile
    from concourse import bacc, mybir

    f32 = mybir.dt.float32
    f16 = mybir.dt.float16
    bf16 = mybir.dt.bfloat16
    f8 = mybir.dt.float8e4
    AF = mybir.ActivationFunctionType
    OP = mybir.AluOpType
    AX = mybir.AxisListType

    nc = bacc.Bacc("TRN2", target_bir_lowering=False, debug=False)

    xin = nc.dram_tensor("xin", [NK, 128, NCH, CK], f16, kind="ExternalInput").ap()
    blobR = nc.dram_tensor("blobR", [128, _R_COLS], f32, kind="ExternalInput").ap()
    blobW1 = nc.dram_tensor("blobW1", [128, 1536], f32, kind="ExternalInput").ap()
    blobL = nc.dram_tensor("blobL", [128, _L_COLS], f32, kind="ExternalInput").ap()
    blob2 = nc.dram_tensor("blob2", [16, _B2_COLS], f32, kind="ExternalInput").ap()
    yout = nc.dram_tensor("yout", [NK, 128, NCH, CK], f16, kind="ExternalOutput").ap()

    with tile.TileContext(nc) as tc:
        with ExitStack() as ctx:
            const = ctx.enter_context(tc.tile_pool(name="const", bufs=1))
            pers = ctx.enter_context(tc.tile_pool(name="pers", bufs=1))
            xp = ctx.enter_context(tc.tile_pool(name="xp", bufs=NK))
            vp = ctx.enter_context(tc.tile_pool(name="vp", bufs=1, space="PSUM"))
            vsb = ctx.enter_context(tc.tile_pool(name="vsb", bufs=NK))

            pro = ExitStack()
            pp = pro.enter_context(tc.tile_pool(name="pp", bufs=3, space="PSUM"))
            wps = pro.enter_context(tc.tile_pool(name="wps", bufs=1, space="PSUM"))

            # ---- engine warmup (before consts land) ----
            # DVE zeroes a dummy tile; PE chews matmuls on it so the PE
            # clock is fully ramped when real work arrives; ACT preloads
            # the sqrt table (covers sqrt/square/relu/copy).
            wtile = pers.tile([128, CK], f16, name="wtile")
            nc.vector.memset(wtile, 0.0)
            wsc = pers.tile([1, 4], f32, name="wsc")
            nc.scalar.activation(wsc, wtile[:1, :4], AF.Sqrt)

            # ---- const loads ----
            blobR_sb = const.tile([128, _R_COLS], f32, name="blobR_sb")
            nc.sync.dma_start(out=blobR_sb, in_=blobR)
            blob2_sb = const.tile([16, _B2_COLS], f32, name="blob2_sb")
            nc.sync.dma_start(out=blob2_sb, in_=blob2)
            blobW1_sb = const.tile([128, 1536], f32, name="blobW1_sb")
            nc.scalar.dma_start(out=blobW1_sb, in_=blobW1)
            blobL_sb = const.tile([128, _L_COLS], f32, name="blobL_sb")
            nc.scalar.dma_start(out=blobL_sb, in_=blobL)

            teT_sb = blobR_sb[:, 0:96].rearrange("p (c j) -> p c j", c=6)
            curT_sb = blobR_sb[:, 96:102].rearrange("p (c j) -> p c j", c=6)
            W4T_sb = blobR_sb[:, 102:103]
            M8_sb = blobR_sb[:, 103:119]
            oh_sb = blobR_sb[:, 119:120]
            W2T_sb = blobR_sb[:, 120:632].bitcast(bf16).rearrange(
                "p (c j) -> p c j", c=4)
            W3T_sb = blobR_sb[:, 632:760].bitcast(bf16).rearrange(
                "p (c j) -> p c j", c=2)
            combT_sb = blobR_sb[:, 760:808].bitcast(f8).rearrange(
                "p (c j) -> p c j", c=12)
            W1T_sb = blobW1_sb.bitcast(f8).rearrange("p (c j) -> p c j", c=12)
            laGT_sb = blobL_sb[:, 0:384].bitcast(f16).rearrange(
                "p (c j) -> p c j", c=6)
            lbG_sb = blobL_sb[:, 384:768].bitcast(f16)
            ident128_sb = blobL_sb[:, 768:832].bitcast(f16)

            te_row_sb = blob2_sb[:, 0:768]
            E16_sb = blob2_sb[:, 768:896]
            ident_sb = blob2_sb[:, 896:912]
            cur_row_sb = blob2_sb[:1, 912:1680]
            ones16_sb = blob2_sb[:1, 1680:1696]
            ones128_sb = blob2_sb[:1, 1696:1824]
            ones16b_sb = blob2_sb[:1, 1824:1832].bitcast(bf16)
            b1_sb = blob2_sb[:1, 1832:2088].bitcast(bf16)
            b2_sb = blob2_sb[:1, 2088:2216].bitcast(bf16)
            b3_sb = blob2_sb[:1, 2216:2280].bitcast(bf16)
            b4_sb = blob2_sb[:1, 2280:2281].bitcast(bf16)[:, 0:1]

            # ---- x-in DMAs (8 chunk DMAs on the sync ring, after consts) ----
            xhs = []
            for k in range(NK):
                xh = xp.tile([128, NCH, CK], f16, tag="xh", name=f"xh{k}")
                nc.sync.dma_start(out=xh, in_=xin[k])
                xhs.append(xh)

            # ---- PE p-state warmup on the dummy tile ----
            w_ps = wps.tile([16, CK], f32, tag="wps", name="w_ps")
            for _ in range(N_WARM_MM):
                nc.tensor.matmul(w_ps, lhsT=wtile[:, 0:16], rhs=wtile,
                                 start=True, stop=True)

            v_sbs = {}

            def emit_v(k):
                v_ps = vp.tile([128, CK], f32, tag="v", name="v_ps")
                for c in range(NCH):
                    nc.tensor.matmul(v_ps, lhsT=laGT_sb[:, c, :],
                                     rhs=xhs[k][:, c, :],
                                     start=(c == 0), stop=(c == NCH - 1))
                v_sb = vsb.tile([128, CK], f16, tag="v_sb", name=f"v{k}")
                nc.scalar.copy(v_sb, v_ps)
                v_sbs[k] = v_sb

            # ================= routing prologue (replicated) =================
            # dots[n] = te[n] . cur
            dots_ps = pp.tile([16, 1], f32, tag="pp")
            for c in range(NCH):
                nc.tensor.matmul(dots_ps, lhsT=teT_sb[:, c, :], rhs=curT_sb[:, c, :],
                                 start=(c == 0), stop=(c == NCH - 1))
            dots = pers.tile([16, 1], f32)
            nc.scalar.copy(dots, dots_ps)

            # similarity MLP layer 1 (fp8)
            h1_ps = pp.tile([16, 512], f32, tag="pp")
            for c in range(12):
                nc.tensor.matmul(h1_ps, lhsT=combT_sb[:, c, :], rhs=W1T_sb[:, c, :],
                                 start=(c == 0), stop=False)
            nc.tensor.matmul(h1_ps, lhsT=ones16b_sb, rhs=b1_sb, start=False, stop=True)
            h1 = pers.tile([16, 512], f32)
            nc.scalar.activation(h1, h1_ps, AF.Relu)

            # norms (DVE, keeps ACT free) + cur2 broadcast
            scr_te = pers.tile([16, H], f32)
            te2 = pers.tile([16, 1], f32)
            nc.vector.scalar_tensor_tensor(scr_te, in0=te_row_sb, scalar=1.0,
                                           in1=te_row_sb, op0=OP.mult, op1=OP.mult,
                                           accum_out=te2)
            scr_cur = pers.tile([1, H], f32)
            cur2 = pers.tile([1, 1], f32)
            nc.vector.scalar_tensor_tensor(scr_cur, in0=cur_row_sb, scalar=1.0,
                                           in1=cur_row_sb, op0=OP.mult, op1=OP.mult,
                                           accum_out=cur2)
            c2b_ps = pp.tile([16, 1], f32, tag="pp")
            nc.tensor.matmul(c2b_ps, lhsT=ones16_sb, rhs=cur2, start=True, stop=True)
            c2b = pers.tile([16, 1], f32)
            nc.scalar.copy(c2b, c2b_ps)

            # euclid distance pieces on DVE (needs dots/te2/c2b only)
            e2 = pers.tile([16, 1], f32)
            nc.vector.scalar_tensor_tensor(e2, in0=dots, scalar=-2.0, in1=te2,
                                           op0=OP.mult, op1=OP.add)
            nc.vector.tensor_add(e2, e2, c2b)
            nc.vector.tensor_scalar_max(e2, e2, 0.0)

            # sqrt block early (same ACT table as relu/square/copy), then a
            # dummy sigmoid gated on eu prefetches the sigmoid table; every
            # ACT op after it (relu/copy/sigmoid) lives in the sigmoid set.
            emb_n = pers.tile([16, 1], f32)
            nc.scalar.sqrt(emb_n, te2)
            curn16 = pers.tile([16, 1], f32)
            nc.scalar.sqrt(curn16, c2b)
            curn = pers.tile([1, 1], f32)
            nc.scalar.sqrt(curn, cur2)
            eu = pers.tile([16, 1], f32)
            nc.scalar.sqrt(eu, e2)
            den = pers.tile([16, 1], f32)
            nc.vector.tensor_mul(den, emb_n, curn16)
            nc.vector.tensor_scalar_max(den, den, 1e-8)
            rden = pers.tile([16, 1], f32)
            nc.vector.reciprocal(rden, den)
            eup1 = pers.tile([16, 1], f32)
            nc.vector.tensor_scalar_add(eup1, eu, 1.0)
            nc.scalar.activation(wsc[:1, :1], eu[:1, 0:1], AF.Sigmoid)

            h1T = pers.tile([128, 4, 16], bf16)
            for c in range(4):
                tr_ps = pp.tile([128, 16], f32, tag="pp")
                nc.tensor.transpose(tr_ps, h1[:, c * 128:(c + 1) * 128], ident_sb)
                nc.scalar.copy(h1T[:, c, :], tr_ps)
            h2_ps = pp.tile([16, 256], f32, tag="pp")
            for c in range(4):
                nc.tensor.matmul(h2_ps, lhsT=h1T[:, c, :], rhs=W2T_sb[:, c, :],
                                 start=(c == 0), stop=False)
            nc.tensor.matmul(h2_ps, lhsT=ones16b_sb, rhs=b2_sb, start=False, stop=True)
            h2 = pers.tile([16, 256], f32)
            nc.scalar.activation(h2, h2_ps, AF.Relu)
            h2T = pers.tile([128, 2, 16], bf16)
            for c in range(2):
                tr_ps = pp.tile([128, 16], f32, tag="pp")
                nc.tensor.transpose(tr_ps, h2[:, c * 128:(c + 1) * 128], ident_sb)
                nc.scalar.copy(h2T[:, c, :], tr_ps)
            h3_ps = pp.tile([16, 128], f32, tag="pp")
            for c in range(2):
                nc.tensor.matmul(h3_ps, lhsT=h2T[:, c, :], rhs=W3T_sb[:, c, :],
                                 start=(c == 0), stop=False)
            nc.tensor.matmul(h3_ps, lhsT=ones16b_sb, rhs=b3_sb, start=False, stop=True)
            h3 = pers.tile([16, 128], f32)
            nc.scalar.activation(h3, h3_ps, AF.Relu)
            h3T = pers.tile([128, 16], f32)
            tr_ps = pp.tile([128, 16], f32, tag="pp")
            nc.tensor.transpose(tr_ps, h3, ident_sb)
            nc.scalar.copy(h3T, tr_ps)


            z4_ps = pp.tile([16, 1], f32, tag="pp")
            nc.tensor.matmul(z4_ps, lhsT=h3T, rhs=W4T_sb, start=True, stop=False)
            nc.tensor.matmul(z4_ps, lhsT=ones16b_sb, rhs=b4_sb, start=False, stop=True)
            nn_sim = pers.tile([16, 1], f32)
            nc.scalar.activation(nn_sim, z4_ps, AF.Sigmoid)
            emit_v(0)

            # cos / euclid sims (DVE)
            cos = pers.tile([16, 1], f32)
            nc.vector.tensor_mul(cos, dots, rden)
            es = pers.tile([16, 1], f32)
            nc.vector.reciprocal(es, eup1)

            sims16 = pers.tile([16, 1], f32)
            nc.vector.scalar_tensor_tensor(sims16, in0=cos, scalar=0.4 / 0.3, in1=es,
                                           op0=OP.mult, op1=OP.add)
            nc.vector.tensor_add(sims16, sims16, nn_sim)
            nc.vector.tensor_scalar_mul(sims16, sims16, 0.3)
            sr_ps = pp.tile([1, 16], f32, tag="pp")
            nc.tensor.transpose(sr_ps, sims16, ident_sb)
            sims_row = pers.tile([1, 16], f32)
            nc.scalar.copy(sims_row, sr_ps)
            emit_v(1)

            # ---- top-3 threshold (DVE; overlaps v on PE) ----
            m1 = pers.tile([1, 1], f32)
            nc.vector.reduce_max(m1, sims_row, axis=AX.X)
            msk = pers.tile([1, 16], f32)
            nc.vector.tensor_scalar(msk, in0=sims_row, scalar1=m1, scalar2=None, op0=OP.is_ge)
            s2 = pers.tile([1, 16], f32)
            nc.vector.scalar_tensor_tensor(s2, in0=msk, scalar=-1e30, in1=sims_row,
                                           op0=OP.mult, op1=OP.add)
            m2 = pers.tile([1, 1], f32)
            nc.vector.reduce_max(m2, s2, axis=AX.X)
            msk2 = pers.tile([1, 16], f32)
            nc.vector.tensor_scalar(msk2, in0=s2, scalar1=m2, scalar2=None, op0=OP.is_ge)
            s3 = pers.tile([1, 16], f32)
            nc.vector.scalar_tensor_tensor(s3, in0=msk2, scalar=-1e30, in1=s2,
                                           op0=OP.mult, op1=OP.add)
            m3 = pers.tile([1, 1], f32)
            nc.vector.reduce_max(m3, s3, axis=AX.X)
            ge3 = pers.tile([1, 16], f32)
            nc.vector.tensor_scalar(ge3, in0=sims_row, scalar1=m3, scalar2=None, op0=OP.is_ge)
            pos = pers.tile([1, 16], f32)
            nc.vector.tensor_scalar(pos, in0=sims_row, scalar1=0.0, scalar2=None, op0=OP.is_gt)
            m12 = pers.tile([1, 16], f32)
            nc.vector.tensor_mul(m12, ge3, pos)
            w_row = pers.tile([1, 16], f32)
            total = pers.tile([1, 1], f32)
            nc.vector.scalar_tensor_tensor(w_row, in0=m12, scalar=1.0, in1=sims_row,
                                           op0=OP.mult, op1=OP.mult, accum_out=total)
            tpos = pers.tile([1, 1], f32)
            nc.vector.tensor_scalar(tpos, in0=total, scalar1=0.0, scalar2=None, op0=OP.is_gt)
            tm1 = pers.tile([1, 1], f32)
            nc.vector.tensor_scalar_add(tm1, total, -1.0)
            safe = pers.tile([1, 1], f32)
            nc.vector.scalar_tensor_tensor(safe, in0=tm1, scalar=tpos, in1=ones16_sb[:, 0:1],
                                           op0=OP.mult, op1=OP.add)
            rinv = pers.tile([1, 1], f32)
            nc.vector.reciprocal(rinv, safe)
            wn_row = pers.tile([1, 16], f32)
            nc.vector.tensor_scalar_mul(wn_row, w_row, rinv)

            # fusion coefficients
            fw = pers.tile([1, 1], f32)
            nc.vector.tensor_scalar(fw, in0=curn, scalar1=0.1, scalar2=0.5,
                                    op0=OP.mult, op1=OP.min)
            cc = pers.tile([1, 2], f32)   # [c2*S | c1*S]
            c2v = pers.tile([1, 1], f32)
            nc.vector.tensor_mul(c2v, fw, tpos)
            nc.vector.tensor_scalar_mul(cc[:, 0:1], c2v, SCALING)
            nc.vector.tensor_scalar(cc[:, 1:2], in0=cc[:, 0:1], scalar1=-1.0, scalar2=SCALING,
                                    op0=OP.mult, op1=OP.add)
            ccb_ps = pp.tile([128, 2], f32, tag="pp")
            nc.tensor.matmul(ccb_ps, lhsT=ones128_sb, rhs=cc, start=True, stop=True)
            cc_b = pers.tile([128, 2], f32)
            nc.scalar.copy(cc_b, ccb_ps)

            # wn onto 128 (task,rank) partitions + spread along free dim
            wc_ps = pp.tile([16, 1], f32, tag="pp")
            nc.tensor.transpose(wc_ps, wn_row, ident_sb[:1, :1])
            wn_col = pers.tile([16, 1], f32)
            nc.scalar.copy(wn_col, wc_ps)
            we_ps = pp.tile([128, 1], f32, tag="pp")
            nc.tensor.matmul(we_ps, lhsT=E16_sb, rhs=wn_col, start=True, stop=True)
            wn_ext = pers.tile([128, 1], f32)
            nc.scalar.copy(wn_ext, we_ps)
            # selectors: G2 [128,16] (A-side, scaled) and sc_a (B-side lhsT)
            sc_a = pers.tile([128, 16], f16)
            nc.vector.tensor_scalar_mul(sc_a[:, 0:8], M8_sb[:, 0:8], oh_sb)
            nc.vector.tensor_scalar_mul(sc_a[:, 8:16], M8_sb[:, 8:16], wn_ext)
            G2f = pers.tile([128, 16], f16)
            nc.vector.tensor_scalar(G2f[:, 0:8], in0=sc_a[:, 0:8], scalar1=cc_b[:, 1:2],
                                    scalar2=None, op0=OP.mult)
            nc.vector.tensor_scalar(G2f[:, 8:16], in0=sc_a[:, 8:16], scalar1=cc_b[:, 0:1],
                                    scalar2=None, op0=OP.mult)

            bc_ps = pp.tile([16, H], f32, tag="bc", bufs=1)
            nc.tensor.matmul(bc_ps[:, 0:512], lhsT=sc_a, rhs=lbG_sb[:, 0:512],
                             start=True, stop=True)
            nc.tensor.matmul(bc_ps[:, 512:768], lhsT=sc_a, rhs=lbG_sb[:, 512:768],
                             start=True, stop=True)
            B_comb = pers.tile([16, H], f16)
            nc.scalar.copy(B_comb, bc_ps)

            pro.close()

            # ================= main loop =================
            with (
                tc.tile_pool(name="yp", bufs=2) as yp,
                tc.tile_pool(name="usb", bufs=3) as usb,
                tc.tile_pool(name="ups", bufs=2, space="PSUM") as ups,
                tc.tile_pool(name="lps", bufs=5, space="PSUM") as lps,
            ):
                u_sbs = {}

                def emit_u(k):
                    u_ps = ups.tile([16, CK], f32, tag="ups", name="u_ps")
                    nc.tensor.matmul(u_ps, lhsT=G2f, rhs=v_sbs[k],
                                     start=True, stop=True)
                    u_sb = usb.tile([16, CK], f16, tag="usb", name="u_sb")
                    nc.scalar.copy(u_sb, u_ps)
                    u_sbs[k] = u_sb

                def emit_lora(k):
                    yt = yp.tile([128, NCH, CK], f16, tag="yt", name="yt")
                    for c in range(NCH):
                        l_ps = lps.tile([128, CK], f32, tag="lora", name="l_ps")
                        if c < 4:
                            nc.tensor.matmul(l_ps,
                                             lhsT=B_comb[:, c * 128:(c + 1) * 128],
                                             rhs=u_sbs[k], start=True, stop=True)
                            nc.vector.tensor_add(yt[:, c, :], xhs[k][:, c, :], l_ps)
                        else:
                            # x folded in on the PE; ACT does the PSUM drain
                            nc.tensor.matmul(l_ps,
                                             lhsT=B_comb[:, c * 128:(c + 1) * 128],
                                             rhs=u_sbs[k], start=True, stop=False)
                            nc.tensor.matmul(l_ps, lhsT=ident128_sb,
                                             rhs=xhs[k][:, c, :],
                                             start=False, stop=True)
                            nc.scalar.copy(yt[:, c, :], l_ps)
                    oeng = nc.scalar if k % 2 == 0 else nc.gpsimd
                    oeng.dma_start(out=yout[k], in_=yt)

                emit_u(0)
                emit_u(1)
                emit_lora(0)
                emit_v(2)
                emit_u(2)
                emit_lora(1)
                emit_v(3)
                emit_u(3)
                emit_lora(2)
                emit_v(4)
                emit_u(4)
                emit_lora(3)
                emit_v(5)
                emit_u(5)
                emit_lora(4)
                emit_v(6)
                emit_u(6)
                emit_lora(5)
                emit_v(7)
                emit_u(7)
                emit_lora(6)
                emit_lora(7)

    nc.compile()
    return nc


def _get_program():
    global _PROGRAM
    if _PROGRAM is None:
        _PROGRAM = _build_program()
    return _PROGRAM


def _chunkpack(a):
    # [C*128, J] -> [128, C*J] so blob[p, c*J+j] = a[c*128+p, j]
    C = a.shape[0] // 128
    return a.reshape(C, 128, -1).transpose(1, 0, 2).reshape(128, -1)


def _make_in_maps(inputs):
    import ml_dtypes

    bfd = ml_dtypes.bfloat16
    f8d = ml_dtypes.float8_e4m3fn

    def bfpack(a):
        return np.ascontiguousarray(a.astype(bfd)).view(np.float32)

    def f16pack(a):
        return np.ascontiguousarray(a.astype(np.float16)).view(np.float32)

    def f8pack(a):
        return np.ascontiguousarray(a.astype(f8d)).view(np.float32)

    hs = np.ascontiguousarray(np.asarray(inputs["hidden_states"], np.float32))
    cur = np.ascontiguousarray(np.asarray(inputs["task_embedding"], np.float32))
    la = np.ascontiguousarray(np.asarray(inputs["loras_a"], np.float32))
    lb = np.ascontiguousarray(np.asarray(inputs["loras_b"], np.float32))
    te = np.ascontiguousarray(np.asarray(inputs["task_embeds"], np.float32))
    W1 = np.asarray(inputs["W1"], np.float32)
    W2 = np.asarray(inputs["W2"], np.float32)
    W3 = np.asarray(inputs["W3"], np.float32)
    W4 = np.asarray(inputs["W4"], np.float32)
    b1 = np.asarray(inputs["b1"], np.float32)
    b2 = np.asarray(inputs["b2"], np.float32)
    b3 = np.asarray(inputs["b3"], np.float32)
    b4 = np.asarray(inputs["b4"], np.float32)
    tid = int(np.asarray(inputs["current_task_id"]))

    idx = np.arange(NR)
    n_idx, r_idx = idx // R, idx % R
    M8 = np.zeros((NR, N_TASKS), np.float32)
    for j in range(N_TASKS):
        M8[:, j] = (r_idx == (j % R)).astype(np.float32)
    E16 = np.zeros((N_TASKS, NR), np.float32)
    E16[n_idx, idx] = 1.0
    onehot_ext = (n_idx == tid).astype(np.float32).reshape(NR, 1)

    comb = np.concatenate([np.repeat(cur[:, None], N_TASKS, axis=1), te.T], axis=0)
    blobR = np.concatenate([
        _chunkpack(np.ascontiguousarray(te.T)),               # 96  teT
        cur.reshape(6, 128).T,                                # 6   curT
        np.ascontiguousarray(W4.T),                           # 1   W4T
        M8,                                                   # 16
        onehot_ext,                                           # 1
        bfpack(_chunkpack(np.ascontiguousarray(W2.T))),       # 512 W2T (bf16)
        bfpack(_chunkpack(np.ascontiguousarray(W3.T))),       # 128 W3T (bf16)
        f8pack(_chunkpack(comb)),                             # 48  combT (fp8)
    ], axis=1).astype(np.float32)
    assert blobR.shape == (128, _R_COLS), blobR.shape

    blobW1 = f8pack(_chunkpack(np.ascontiguousarray(W1.T)))
    assert blobW1.shape == (128, 1536), blobW1.shape

    blobL = np.concatenate([
        f16pack(_chunkpack(np.ascontiguousarray(la.reshape(NR, H).T))),  # 384
        f16pack(lb.transpose(0, 2, 1).reshape(NR, H)),        # 384 lbG
        f16pack(np.eye(128, dtype=np.float32)),               # 64  ident128
    ], axis=1).astype(np.float32)
    assert blobL.shape == (128, _L_COLS), blobL.shape

    def row0(a, n):
        b = np.zeros((16, n), np.float32)
        b[0, :] = a.reshape(-1)
        return b

    bfrow = np.concatenate([
        np.ones(16, np.float32), b1, b2, b3, b4, np.zeros(1, np.float32)])
    bfrow = np.ascontiguousarray(bfrow.astype(bfd)).view(np.float32)
    blob2 = np.concatenate([
        te,                                                   # 768
        E16,                                                  # 128
        np.eye(16, dtype=np.float32),                         # 16
        row0(cur, 768),
        row0(np.ones(16, np.float32), 16),
        row0(np.ones(NR, np.float32), 128),
        row0(bfrow, 457),                                     # bf16: ones|b1..b4
    ], axis=1).astype(np.float32)
    assert blob2.shape == (16, _B2_COLS), blob2.shape

    rep = {"blobR": blobR, "blobW1": blobW1, "blobL": blobL, "blob2": blob2}

    x2 = hs.reshape(B * S, H)
    in_maps = []
    for i in range(NCORES):
        shard = x2[i * TPC:(i + 1) * TPC]                     # [TPC, H]
        xpk = shard.reshape(NK, CK, NCH, 128).transpose(0, 3, 2, 1)
        in_maps.append({"xin": np.ascontiguousarray(xpk.astype(np.float16)),
                        **rep})
    return in_maps


def _unpack_core_y(yarr):
    # [NK, 128, NCH, CK] fp16 -> [TPC, H] f32
    return np.ascontiguousarray(
        yarr.transpose(0, 3, 2, 1).astype(np.float32)).reshape(TPC, H)


def kernel(**inputs):
    from concourse.bass_utils import run_bass_kernel_spmd

    nc = _get_program()
    in_maps = _make_in_maps(inputs)
    res = run_bass_kernel_spmd(nc, in_maps, core_ids=list(range(NCORES)))
    out = np.empty((B * S, H), np.float32)
    for i, r in enumerate(res.results):
        out[i * TPC:(i + 1) * TPC] = _unpack_core_y(r["yout"])
    return out.reshape(B, S, H)
